# revision 30
# baseline (speedup 1.0000x reference)
"""Causal multi-head attention (B=4, H=16, S=2048, D=64) on 8 TRN2 NeuronCores.

Sharding: B*H = 64 heads, 8 heads per core (data/head parallel, no comms).

Host side: Q,K are pre-transposed to d-major [D, S] and pre-cast to bf16
(the device matmuls run bf16 anyway), V is pre-cast to bf16. This removes
all on-device transposes/casts; each head's prep is just three DMAs.

Per-core pipeline (per head):
  - DMA qT,kT [64, 2048] bf16 into the bottom half of zero-padded
    [128, 2048] SBUF tiles (K=128 contraction; the (128,128) PE tile path
    is faster than (64,128)); DMA V into v_aug[:, :, 0:64] with a ones
    column at d=64 (set once per pool slot) for the softmax denominator
  - QK^T strips E^T[k, q] as N<=512 K=128 matmuls into PSUM pieces
  - causal masking of the diagonal tile via a PSUM pre-bias matmul
    (-BIG*I)^T @ tril_strict, so exp emits ~0 below the diagonal
  - exp pieces split between ScalarE (exact, table exp) and VectorE
    (Schraudolph fast-exp: bitcast_bf16(int16(x*FXA + FXB)), ~4% pointwise,
    interleaved per piece so end-to-end error stays ~6e-3)
  - A@V with the ones-column: O[q,0:64] = sum_k A^T_k.T @ V_k, O[q,64] =
    denominator; 2-strip lag keeps the PE from waiting on exp; groups of
    four q-tiles share one PSUM bank + one batched normalize (DVE
    reciprocal + broadcast multiply), then stream out via DMA
"""

import os
import sys

try:
    import concourse.bass as bass  # noqa: F401
except ImportError:
    sys.path.insert(0, "/opt/trn_rl_repo")

import numpy as np

import concourse.mybir as mybir
import concourse.tile as tile
from concourse import bacc
from concourse.bass_utils import run_bass_kernel_spmd
from concourse.masks import make_identity

B, H, S, D = 4, 16, 2048, 64
N_CORES = 8
HEADS = B * H
HPC = HEADS // N_CORES  # heads per core
P = 128
ST = S // P  # 16 s-tiles per head

F32 = mybir.dt.float32
BF16 = mybir.dt.bfloat16
I16 = mybir.dt.int16

SCALE = 1.0 / float(np.sqrt(D))

LAG = int(os.environ.get("K_LAG", "2"))      # A@V emission lag (strips)
XLAG = int(os.environ.get("K_XLAG", "5"))    # extra lag for late (big) A@V waves
ETBUFS = int(os.environ.get("K_ETBUFS", "3"))
# mask modes: "pool" (default) zeroes the exp output's lower triangle on
# GpSimd -- deterministic-clean across runs. "dve" does the same on
# VectorE. "pe"/"pe2" fold the causal bias into the QK^T PSUM via an
# extra matmul; BOTH showed intermittent slot-A-head corruption on
# hardware (PSUM accumulate groups racing the et-tile rotation) -- do
# not ship them.
MASK_ENG = os.environ.get("K_MASK_ENG", "pool")
DVE_SHARE = float(os.environ.get("K_DVESHARE", "0.40"))  # exp cols on DVE
PIECE = int(os.environ.get("K_PIECE", "1024"))
BIG = 240.0  # causal mask pre-bias; exp((E-240)/8) ~ 1e-13 = zero

# Schraudolph fast-exp on DVE: exp(s*x) ~= bitcast_bf16(int16(x*FXA + FXB)).
FXA = SCALE * 128.0 * float(np.log2(np.e))
FXB = 128.0 * 127.0 - 7.0


def build_nc(heads_per_core=HPC):
    nc = bacc.Bacc("TRN2", target_bir_lowering=False, debug=False,
                   num_devices=N_CORES)
    qt_d = nc.dram_tensor("QT", [heads_per_core, D, S], BF16, kind="ExternalInput")
    kt_d = nc.dram_tensor("KT", [heads_per_core, D, S], BF16, kind="ExternalInput")
    v_d = nc.dram_tensor("V", [heads_per_core, S, D], BF16, kind="ExternalInput")
    o_d = nc.dram_tensor("out", [heads_per_core, S, D], F32, kind="ExternalOutput")

    with tile.TileContext(nc) as tc:
        with (
            tc.tile_pool(name="const", bufs=1) as const,
            tc.tile_pool(name="tp", bufs=2) as tpool,
            tc.tile_pool(name="vp", bufs=2) as vpool,
            tc.tile_pool(name="atp", bufs=int(os.environ.get("K_ATBUFS", "2"))) as atp,
            tc.tile_pool(name="osb", bufs=2) as osbp,
            tc.tile_pool(name="small", bufs=8) as small,
            tc.tile_pool(name="ps", bufs=1, space="PSUM") as ps,
        ):
            trimask = None
            neg_ident = tril_strict = None
            if MASK_ENG == "dve":
                # upper-triangular (incl. diagonal) ones: keep q >= k
                trimask = const.tile([P, P], BF16, tag="trimask")
                nc.gpsimd.memset(trimask, 1.0)
                nc.gpsimd.affine_select(
                    out=trimask, in_=trimask,
                    compare_op=mybir.AluOpType.is_ge,
                    fill=0.0, base=0,
                    pattern=[[1, P]], channel_multiplier=-1,
                )
            elif MASK_ENG in ("pe", "pe2"):
                # mask via PSUM pre-bias: et[k, q] += -BIG where k > q,
                # emitted as one PE matmul (-BIG*I)^T @ tril_strict per
                # strip -- no cross-engine mask hop after exp
                neg_ident = const.tile([P, P], BF16, tag="negid")
                make_identity(nc, neg_ident)
                nc.vector.tensor_scalar_mul(neg_ident, neg_ident, -BIG)
                tril_strict = const.tile([P, P], BF16, tag="trilS")
                nc.gpsimd.memset(tril_strict, 1.0)
                # keep 1 where c > q  (affine value c - q - 1 >= 0)
                nc.gpsimd.affine_select(
                    out=tril_strict, in_=tril_strict,
                    compare_op=mybir.AluOpType.is_ge,
                    fill=0.0, base=-1,
                    pattern=[[-1, P]], channel_multiplier=1,
                )
            # greedy engine split for exp pieces (columns to DVE fast-exp)
            exp_cols = {"dve": 0.0, "tot": 0.0}

            def emit_prep(h, nsplit=1):
                """DMA head h's pre-transposed operands. qT/kT tiles are
                [128, S] with rows 64:128 zeroed once per pool slot
                (contraction padding). v_aug's ones column likewise."""
                qT = tpool.tile([P, S], BF16, tag="qT")
                kT = tpool.tile([P, S], BF16, tag="kT")
                v_aug = vpool.tile([P, ST, D + 1], BF16, tag="vaug")
                if h < 2:  # pool slots keep these across head rotations
                    nc.gpsimd.memset(qT[64:P, :], 0.0)
                    nc.gpsimd.memset(kT[64:P, :], 0.0)
                    nc.gpsimd.memset(v_aug[:, :, D:D + 1], 1.0)
                splits = [(S * i // nsplit, S * (i + 1) // nsplit)
                          for i in range(nsplit)]
                for c0, c1 in splits:
                    nc.sync.dma_start(out=qT[0:D, c0:c1], in_=qt_d[h][:, c0:c1])
                    nc.sync.dma_start(out=kT[0:D, c0:c1], in_=kt_d[h][:, c0:c1])
                nc.sync.dma_start(
                    out=v_aug[:, :, 0:D],
                    in_=v_d[h].rearrange("(b p) d -> p b d", p=P))
                return qT, kT, v_aug

            # Per-head pipeline state, keyed by head; two heads live at once.
            state = {}

            def emit_strip(h, j):
                """QK^T strip j of head h, causal pre-bias, exp."""
                st = state[h]
                qT, kT = st["qT"], st["kT"]
                W = S - P * j  # valid q columns for key-tile j
                at = atp.tile([P, W], BF16, tag=f"at{j}", name=f"at_{h}_{j}")
                st["strips"].append(at)

                off = 0
                pieces = []
                while off < W:
                    w = min(PIECE, W - off)
                    et = ps.tile([P, w], F32, tag="et", bufs=ETBUFS, name="et")
                    # chunk boundaries; under "pe2" the diagonal tile's
                    # accumulation group opens with the causal-bias matmul
                    # and the 128-col data chunk closes it
                    if off == 0 and MASK_ENG == "pe2":
                        nc.tensor.matmul(
                            et[:, 0:P],
                            lhsT=neg_ident, rhs=tril_strict,
                            start=True, stop=False,
                        )
                        nc.tensor.matmul(
                            et[:, 0:P],
                            lhsT=kT[:, P * j:P * (j + 1)],
                            rhs=qT[:, P * j:P * j + P],
                            start=False, stop=True,
                        )
                        c0 = P
                    else:
                        c0 = 0
                    while c0 < w:
                        ce = min((c0 // 512 + 1) * 512, w)
                        qg = P * j + off + c0
                        nc.tensor.matmul(
                            et[:, c0:ce],
                            lhsT=kT[:, P * j:P * (j + 1)],
                            rhs=qT[:, qg:qg + (ce - c0)],
                            start=True, stop=True,
                        )
                        c0 = ce
                    if off == 0 and MASK_ENG == "pe":
                        # accumulate -BIG onto the diagonal tile's lower
                        # triangle before exp reads it (known racy)
                        nc.tensor.matmul(
                            et[:, 0:P],
                            lhsT=neg_ident, rhs=tril_strict,
                            start=False, stop=True,
                            skip_group_check=True,
                        )
                    pieces.append((et, off, w))
                    off += w

                for (et, off, w) in pieces:
                    # interleave exact exp (ACT) with fast-exp (DVE) per
                    # piece: balances the two engines and averages the
                    # fast-exp error across every output row
                    use_dve = (exp_cols["dve"] + w * 0.5
                               < DVE_SHARE * (exp_cols["tot"] + w))
                    exp_cols["tot"] += w
                    if use_dve:
                        exp_cols["dve"] += w
                        nc.vector.tensor_scalar(
                            out=at[:, off:off + w].bitcast(I16),
                            in0=et,
                            scalar1=FXA, scalar2=FXB,
                            op0=mybir.AluOpType.mult,
                            op1=mybir.AluOpType.add,
                        )
                    else:
                        nc.scalar.activation(
                            at[:, off:off + w], et,
                            mybir.ActivationFunctionType.Exp,
                            scale=SCALE,
                        )
                # causal mask inside the diagonal tile: zero where q < k
                # ("pe" variant already handled it as a PSUM pre-bias)
                if MASK_ENG == "dve":
                    nc.vector.tensor_mul(at[:, 0:P], at[:, 0:P], trimask)
                elif MASK_ENG == "pool":
                    nc.gpsimd.affine_select(
                        out=at[:, 0:P], in_=at[:, 0:P],
                        compare_op=mybir.AluOpType.is_ge,
                        fill=0.0, base=0,
                        pattern=[[1, P]], channel_multiplier=-1,
                    )

            def emit_av(h, jq):
                """A@V for q-tile jq of head h (strips 0..jq ready); groups
                of four q-tiles share one PSUM bank + one batched normalize;
                DMA the group out right away."""
                st = state[h]
                strips, v_aug, o_sb = st["strips"], st["v_aug"], st["o_sb"]
                if jq % 4 == 0:
                    st["o4"] = ps.tile([P, 4, D + 1], F32, tag="o",
                                       bufs=2, name="o4")
                o4 = st["o4"]
                for k in range(jq + 1):
                    nc.tensor.matmul(
                        o4[:, jq % 4, :],
                        lhsT=strips[k][:, P * (jq - k):P * (jq - k) + P],
                        rhs=v_aug[:, k, :],
                        start=(k == 0), stop=(k == jq),
                    )
                if jq % 4 == 3:
                    recip4 = small.tile([P, 4], F32, tag="recip")
                    nc.vector.reciprocal(
                        recip4,
                        o4[:, :, D:D + 1].rearrange("p a b -> p (a b)"),
                    )
                    rb = bass.AP(tensor=recip4.tensor, offset=recip4.offset,
                                 ap=[recip4.ap[0], recip4.ap[1], [0, D]])
                    nc.vector.tensor_tensor(
                        out=o_sb[:, jq - 3:jq + 1, :],
                        in0=o4[:, :, 0:D], in1=rb,
                        op=mybir.AluOpType.mult,
                    )
                    nc.sync.dma_start(
                        out=o_d[h].rearrange("(b p) d -> p b d", p=P)
                                  [:, jq - 3:jq + 1, :],
                        in_=o_sb[:, jq - 3:jq + 1, :],
                    )
                if jq == ST - 1:
                    del state[h]

            # One flattened software pipeline over (head, strip): the A@V
            # wave trails the QK^T/exp wave ACROSS head boundaries. Later
            # q-tiles get EXTRA lag: exp work per strip shrinks with j while
            # the A@V train grows with jq, so pushing the big trains into the
            # next head's long-exp slots keeps the exp engines fed.
            tasks = [(h, j) for h in range(heads_per_core) for j in range(ST)]
            av_slot = {}
            for g_av, (h_av, j_av) in enumerate(tasks):
                av_slot[g_av] = g_av + LAG + (XLAG if j_av >= 12 else 0)
            qT0, kT0, v_aug0 = emit_prep(
                0, nsplit=int(os.environ.get("K_NSPLIT0", "2")))
            state[0] = {"qT": qT0, "kT": kT0, "v_aug": v_aug0, "strips": [],
                        "o_sb": osbp.tile([P, ST, D], F32, tag="osb", name="osb0")}
            av_next = 0
            for g, (h, j) in enumerate(tasks):
                emit_strip(h, j)
                if j == 8 and h + 1 < heads_per_core:
                    qTn, kTn, v_augn = emit_prep(h + 1)
                    state[h + 1] = {
                        "qT": qTn, "kT": kTn, "v_aug": v_augn, "strips": [],
                        "o_sb": osbp.tile([P, ST, D], F32, tag="osb",
                                          name=f"osb{h + 1}"),
                    }
                while av_next < len(tasks) and av_slot[av_next] <= g:
                    emit_av(*tasks[av_next])
                    av_next += 1
            while av_next < len(tasks):
                emit_av(*tasks[av_next])
                av_next += 1

    nc.compile()
    return nc


_NC_CACHE = {}


def _get_nc(heads_per_core=HPC):
    if heads_per_core not in _NC_CACHE:
        _NC_CACHE[heads_per_core] = build_nc(heads_per_core)
    return _NC_CACHE[heads_per_core]


def run_sharded(Q, K, V, heads_per_core=HPC, **run_kwargs):
    """Q, K, V: [HEADS-or-subset, S, D] f32 flattened over (B, H)."""
    import ml_dtypes
    nc = _get_nc(heads_per_core)
    n = heads_per_core
    bf = ml_dtypes.bfloat16
    # host-side layout: d-major bf16 Q^T/K^T, bf16 V
    QT = np.ascontiguousarray(Q.transpose(0, 2, 1)).astype(bf)
    KT = np.ascontiguousarray(K.transpose(0, 2, 1)).astype(bf)
    Vb = V.astype(bf)
    in_maps = [
        {
            "QT": np.ascontiguousarray(QT[i * n:(i + 1) * n]),
            "KT": np.ascontiguousarray(KT[i * n:(i + 1) * n]),
            "V": np.ascontiguousarray(Vb[i * n:(i + 1) * n]),
        }
        for i in range(N_CORES)
    ]
    last_err = None
    for attempt in range(3):
        try:
            res = run_bass_kernel_spmd(nc, in_maps,
                                       core_ids=list(range(N_CORES)),
                                       **run_kwargs)
            out = np.concatenate(
                [np.asarray(res.results[i]["out"]) for i in range(N_CORES)],
                axis=0)
            return out, res
        except Exception as e:  # transient NRT_EXEC_UNIT_UNRECOVERABLE etc.
            last_err = e
            import time
            time.sleep(2.0)
    raise last_err


def kernel(Q, K, V, mask=None):
    Q = np.asarray(Q, dtype=np.float32).reshape(HEADS, S, D)
    K = np.asarray(K, dtype=np.float32).reshape(HEADS, S, D)
    V = np.asarray(V, dtype=np.float32).reshape(HEADS, S, D)
    out, _ = run_sharded(Q, K, V)
    return out.reshape(B, H, S, D)
